# revision 10
# baseline (speedup 1.0000x reference)
"""Multi-head attention, tensor-parallel over heads x data-parallel over batch.

8 NeuronCores: core c handles batch b=c//2, head-group g=c%2 (8 heads, 512 chans).
Each core computes its head-group's attention + partial output projection;
the two partials per batch are summed on the host (row-parallel Wo unshard).

v3 (all-bf16 matmuls, continuous-PE schedule, DMA choreography):
  - every matmul operand is bf16 (halves DMA + LDWEIGHTS; PSUM stays fp32)
  - softmax 1/sum on VectorE via reciprocal_approx_fast (ScalarE runs ONLY
    Exp -> single ACT table set, no table switching, and the norm no longer
    clogs the ACT FIFO). recip inputs must be partition-offset-0 APs (the
    custom DVE op silently misreads offset sources), hence the [ones|v]
    stationary packing that puts sums on partitions 0:64.
  - merged multi-chunk input DMAs (few queue issues; per-chunk pacing) split
    across Sync and GpSimd queues
  - only pair0's first q/k chains run upfront; every other projection chain
    is interleaved into the attention jb-loops, so TensorE streams without
    gaps and the PE HAM stays un-throttled; out-proj chains for the first
    token half are interleaved into pair 3's attention
  - output DMAs round-robin over 4 queues

Per-core dataflow:
  qT/kT = W.T @ xT        [chan, tok] transposed projections
  v     = xkv @ Wv        [tok, chan] natural projection, stored per head
                          pair as [ones|v_even|ones|v_odd] (64 cols each)
  scoresT[j,i] = k.q      row-group packed pairs -> one [128,1024] psum
  expT  = exp(s*scale + maskbias)   one ScalarE op per (pair, ih, jb), bf16 out
  AV:   lhsT = [ones|v_h] (M=128) -> po rows = [sums replicated | o_h]
  norm: rb = recip(sums) on VectorE, oT = po * rb on VectorE (bf16 out)
  out   = oT.T @ Wo       partial output projection
"""

import numpy as np

import concourse.bacc as bacc
import concourse.mybir as mybir
import concourse.tile as tile
from concourse import bass_utils

B = 4
T = 1024          # tokens (N = L)
D = 1024          # model dim
CH = 64           # channels per head
G = 512           # channels per head-group (8 heads)
SCALE = CH ** -0.5
NEG = -30000.0    # mask bias (exp(x + NEG) == 0)
F32 = mybir.dt.float32
BF16 = mybir.dt.bfloat16
BF_NP = mybir.dt.np(mybir.dt.bfloat16)

N_CORES = 8
KB = 8            # 128-row contraction blocks over D
TB = 8            # 128-token blocks
PAIRS = 4         # head pairs per core
VW = 256          # v-tile columns per head pair: [ones|v_even|ones|v_odd]
AV_LAG = 3        # software-pipeline depth: AV trails scores/exp by this many jb

LAST_RESULTS = None
_CACHE = {}


def _emit(tc):
    nc = tc.nc
    xqT = nc.dram_tensor("xqT", [D, T], BF16, kind="ExternalInput").ap()
    xkvT = nc.dram_tensor("xkvT", [D, T], BF16, kind="ExternalInput").ap()
    wq = nc.dram_tensor("wq", [D, G], BF16, kind="ExternalInput").ap()
    wk = nc.dram_tensor("wk", [D, G], BF16, kind="ExternalInput").ap()
    wv = nc.dram_tensor("wv", [D, G], BF16, kind="ExternalInput").ap()
    wo = nc.dram_tensor("wo", [G, D], BF16, kind="ExternalInput").ap()
    mb = nc.dram_tensor("mb", [128, TB], F32, kind="ExternalInput").ap()
    vones = nc.dram_tensor("vones", [128, PAIRS * 2 * CH], BF16,
                           kind="ExternalInput").ap()
    out = nc.dram_tensor("out", [T, D], F32, kind="ExternalOutput").ap()

    Exp = mybir.ActivationFunctionType.Exp

    with (
        tc.tile_pool(name="wpool", bufs=1) as wpool,
        tc.tile_pool(name="xpool", bufs=1) as xpool,
        tc.tile_pool(name="apool", bufs=1) as apool,
        tc.tile_pool(name="epool", bufs=1) as epool,
        tc.tile_pool(name="npool", bufs=1) as npool,
        tc.tile_pool(name="opool", bufs=4) as opool,
        tc.tile_pool(name="psum", bufs=1, space="PSUM") as psum,
    ):
        # ---------------- input DMA ----------------
        # Big multi-chunk DMAs (one queue issue each, ~860ns/issue) ordered
        # by first use; sync carries the v-projection inputs, gpsimd the
        # q/k/o side, so neither engine's issue stream gates the other.
        mask_t = wpool.tile([128, TB], F32, name="mask_t", tag="mask")
        nc.sync.dma_start(mask_t[:], mb[:])

        # xkv / xq as single [128, KB*T] tiles, DMA'd in 2-chunk pieces so
        # the k-accumulation chains can start before the whole tensor lands
        xkv_b = xpool.tile([128, KB * T], BF16, name="xkv", tag="xkv")
        xq_b = xpool.tile([128, KB * T], BF16, name="xq", tag="xq")
        xkv_src = xkvT.rearrange("(kb p) t -> p kb t", p=128)
        xq_src = xqT.rearrange("(kb p) t -> p kb t", p=128)
        xkv_dst = xkv_b.rearrange("p (kb t) -> p kb t", t=T)
        xq_dst = xq_b.rearrange("p (kb t) -> p kb t", t=T)
        xkv_t = [xkv_b[:, k * T:(k + 1) * T] for k in range(KB)]
        xq_t = [xq_b[:, k * T:(k + 1) * T] for k in range(KB)]

        wv_b = wpool.tile([128, KB * G], BF16, name="wv", tag="wv")
        wq_b = wpool.tile([128, KB * G], BF16, name="wq", tag="wq")
        wk_b = wpool.tile([128, KB * G], BF16, name="wk", tag="wk")
        wv_t = [wv_b[:, k * G:(k + 1) * G] for k in range(KB)]
        wq_t = [wq_b[:, k * G:(k + 1) * G] for k in range(KB)]
        wk_t = [wk_b[:, k * G:(k + 1) * G] for k in range(KB)]

        def w_dma(queue, dst_b, src):
            queue.dma_start(dst_b.rearrange("p (kb g) -> p kb g", g=G),
                            src.rearrange("(kb p) g -> p kb g", p=128))

        # sync: xkv (paced) + wv
        nc.sync.dma_start(xkv_dst[:, 0:2, :], xkv_src[:, 0:2, :])
        w_dma(nc.sync, wv_b, wv)
        nc.sync.dma_start(xkv_dst[:, 2:4, :], xkv_src[:, 2:4, :])
        nc.sync.dma_start(xkv_dst[:, 4:6, :], xkv_src[:, 4:6, :])
        nc.sync.dma_start(xkv_dst[:, 6:8, :], xkv_src[:, 6:8, :])

        # gpsimd: wq/wk + xq (paced) + wo + vones
        w_dma(nc.gpsimd, wq_b, wq)
        nc.gpsimd.dma_start(xq_dst[:, 0:2, :], xq_src[:, 0:2, :])
        w_dma(nc.gpsimd, wk_b, wk)
        nc.gpsimd.dma_start(xq_dst[:, 2:4, :], xq_src[:, 2:4, :])
        nc.gpsimd.dma_start(xq_dst[:, 4:6, :], xq_src[:, 4:6, :])
        nc.gpsimd.dma_start(xq_dst[:, 6:8, :], xq_src[:, 6:8, :])

        wo_b = wpool.tile([128, PAIRS * D], BF16, name="wo", tag="wo")
        wo_t = [wo_b[:, m * D:(m + 1) * D] for m in range(PAIRS)]
        nc.gpsimd.dma_start(wo_b.rearrange("p (m d) -> p m d", d=D),
                            wo.rearrange("(m p) d -> p m d", p=128))

        # v tiles: ones into slots 0 and 2 of each pair block (one DMA per
        # tb: the f=2,c=128 view's last-dim 0:64 slice hits cols {0:64,
        # 128:192} of each 256-col pair block)
        v_t = [apool.tile([128, PAIRS * VW], BF16, name=f"v{tb}",
                          tag=f"v{tb}") for tb in range(TB)]
        vo_src = vones.rearrange("p (pb f c) -> p pb f c", f=2, c=CH)
        for tb in range(TB):
            v2c = v_t[tb].rearrange("p (pb f c) -> p pb f c", f=2, c=128)
            nc.gpsimd.dma_start(v2c[:, :, :, 0:CH], vo_src[:])

        # ---------------- v = xkv @ Wv (natural layout) ----------------
        for tb in range(TB):
            ps = psum.tile([128, 512], F32, name="ps_acc", tag="acc", bufs=2)
            for k in range(KB):
                nc.tensor.matmul(
                    ps[:],
                    xkv_t[k][:, tb * 128:(tb + 1) * 128],
                    wv_t[k][:],
                    start=(k == 0),
                    stop=(k == KB - 1),
                )
            v4 = v_t[tb].rearrange("p (pb four c) -> p pb four c",
                                   four=4, c=CH)
            ps3 = ps.rearrange("p (pb two c) -> p pb two c", two=2, c=CH)
            nc.vector.tensor_copy(v4[:, :, 1:2, :], ps3[:, :, 0:1, :])
            nc.vector.tensor_copy(v4[:, :, 3:4, :], ps3[:, :, 1:2, :])

        # ---------------- qT/kT projection chains ----------------
        qT_t = [apool.tile([128, T], BF16, name=f"qT{m}", tag=f"qT{m}")
                for m in range(PAIRS)]
        kT_t = [apool.tile([128, T], BF16, name=f"kT{m}", tag=f"kT{m}")
                for m in range(PAIRS)]

        def proj_chain(p, which, ic):
            src, w_t, dst = ((xq_t, wq_t, qT_t), (xkv_t, wk_t, kT_t))[which]
            csl = slice(ic * 512, (ic + 1) * 512)
            ps = psum.tile([128, 512], F32, name="ps_acc", tag="acc", bufs=2)
            for k in range(KB):
                nc.tensor.matmul(
                    ps[:],
                    w_t[k][:, p * 128:(p + 1) * 128],
                    src[k][:, csl],
                    start=(k == 0),
                    stop=(k == KB - 1),
                )
            nc.vector.tensor_copy(dst[p][:, csl], ps[:])

        # out-projection chain; tb 0..3 read only the ih=0 half of oT, so
        # they can interleave into pair 3's attention
        oq_cycle = [None, None, None]  # filled below (queues)

        def out_chain(tb, ncx, oT_t):
            tsl = slice(tb * 128, (tb + 1) * 128)
            nsl = slice(ncx * 512, (ncx + 1) * 512)
            ps = psum.tile([128, 512], F32, name="ps_acc", tag="acc", bufs=2)
            for m in range(PAIRS):
                nc.tensor.matmul(
                    ps[:],
                    oT_t[m][:, tsl],
                    wo_t[m][:, nsl],
                    start=(m == 0),
                    stop=(m == PAIRS - 1),
                )
            ot = opool.tile([128, 512], F32, name="ot", tag="ot")
            nc.vector.tensor_copy(ot[:], ps[:])
            q = oq_cycle[(tb * 2 + ncx) % 3]
            q.dma_start(out[tsl, nsl], ot[:])

        # upfront: only what attention(p0, ih0, jb0..3) needs
        proj_chain(0, 0, 0)   # q0 ic0
        proj_chain(0, 1, 0)   # k0 ic0

        # ---------------- attention ----------------
        # fill[(p, ih)]: chain thunks run at jb==2 and jb==5 -- the chain a
        # scores MM depends on is always EMITTED earlier, and the Tensor
        # queue executes in order, so this can never deadlock.
        oT_t = [apool.tile([128, T], BF16, name=f"oT{m}", tag=f"oT{m}")
                for m in range(PAIRS)]
        oq_cycle[:] = [nc.scalar, nc.sync, nc.gpsimd]
        fill = {
            (0, 0): [(0, 1, 1), (0, 0, 1)],   # k0ic1, q0ic1
            (0, 1): [(1, 0, 0), (1, 1, 0)],   # q1ic0, k1ic0
            (1, 0): [(1, 1, 1), (1, 0, 1)],
            (1, 1): [(2, 0, 0), (2, 1, 0)],
            (2, 0): [(2, 1, 1), (2, 0, 1)],
            (2, 1): [(3, 0, 0), (3, 1, 0)],
            (3, 0): [(3, 1, 1), (3, 0, 1)],
            (3, 1): ["out00", "out01"],       # out-proj tb=0 (ih0 tokens)
        }
        for p in range(PAIRS):
            oT = oT_t[p]
            for ih in range(2):
                isl = slice(ih * 512, (ih + 1) * 512)
                po = psum.tile([128, 1024], F32, name="po", tag="po", bufs=1)
                pend = []
                slots = list(fill[(p, ih)])
                for jb in range(TB):
                    jsl = slice(jb * 128, (jb + 1) * 128)
                    # one [128, 1024] psum tile: bank0 = head 2p, bank1 = 2p+1
                    pss = psum.tile([128, 1024], F32, name="ps_s", tag="sc",
                                    bufs=2)
                    for h in (0, 1):
                        hsl = slice(h * 64, (h + 1) * 64)
                        nc.tensor.matmul(
                            pss[:, h * 512:(h + 1) * 512],
                            kT_t[p][hsl, jsl],
                            qT_t[p][hsl, isl],
                        )
                    # one exp for both heads; mask bias is per-partition (= j)
                    et = epool.tile([128, 1024], BF16, name="et", tag="et",
                                    bufs=6)
                    nc.scalar.activation(et[:], pss[:], Exp,
                                         bias=mask_t[:, jb:jb + 1],
                                         scale=SCALE)
                    pend.append((jb, et))
                    if len(pend) > AV_LAG:
                        _av(nc, pend.pop(0), p, po, v_t)
                    if jb in (2, 5) and slots:
                        s = slots.pop(0)
                        if s == "out00":
                            out_chain(0, 0, oT_t)
                        elif s == "out01":
                            out_chain(0, 1, oT_t)
                        else:
                            proj_chain(*s)
                while pend:
                    _av(nc, pend.pop(0), p, po, v_t)
                # normalize. po bank h = [s_h (64 rows) | o_h (64 rows)]
                # rb = 1/s on VectorE (~51 ULP approx; sums are O(1..1e3));
                # recip inputs MUST be partition-offset-0 APs
                rb_e = npool.tile([CH, 512], F32, name="rb_e", tag="rb",
                                  bufs=4)
                nc.vector.reciprocal_approx_fast(rb_e[:], po[0:CH, 0:512])
                nc.vector.tensor_mul(oT[0:CH, isl], po[CH:128, 0:512],
                                     rb_e[:])
                rb_o = npool.tile([CH, 512], F32, name="rb_o", tag="rb",
                                  bufs=4)
                nc.vector.reciprocal_approx_fast(rb_o[:], po[0:CH, 512:1024])
                nc.vector.tensor_mul(oT[CH:128, isl], po[CH:128, 512:1024],
                                     rb_o[:])

        # ---------------- out = oT.T @ Wo (rest) ----------------
        # tb 1..3 need only ih0 norms (all done); tb 4..7 need the p3-ih1
        # norm and run while VectorE finishes it
        for tb in range(1, TB):
            for ncx in range(2):
                out_chain(tb, ncx, oT_t)


def _av(nc, item, p, po, v_t):
    jb, et = item
    for h in (0, 1):
        # head 2p+h stationary: [ones|v_h] -> out rows [sums | o_h]
        csl = slice(p * VW + h * 128, p * VW + h * 128 + 128)
        nc.tensor.matmul(
            po[:, h * 512:(h + 1) * 512],
            v_t[jb][:, csl],
            et[:, h * 512:(h + 1) * 512],
            start=(jb == 0),
            stop=(jb == TB - 1),
        )


def build_nc():
    nc = bacc.Bacc("TRN2", target_bir_lowering=False, debug=False,
                   num_devices=N_CORES)
    with tile.TileContext(nc) as tc:
        _emit(tc)
    nc.compile()
    return nc


def _get_compiled():
    if "nc" not in _CACHE:
        _CACHE["nc"] = build_nc()
    return _CACHE["nc"]


def make_in_maps(x_q, x_kv, pad_mask):
    ones = np.ones((128, PAIRS * 2 * CH), BF_NP)
    in_maps = []
    for c in range(N_CORES):
        b, g = divmod(c, 2)
        gs = slice(g * G, (g + 1) * G)
        mbias = np.where(pad_mask[b], np.float32(NEG), np.float32(0.0))
        in_maps.append({
            "xqT": np.ascontiguousarray(x_q[b].T).astype(BF_NP),
            "xkvT": np.ascontiguousarray(x_kv[b].T).astype(BF_NP),
            "wq": _W["q"][:, gs].astype(BF_NP),
            "wk": _W["k"][:, gs].astype(BF_NP),
            "wv": _W["v"][:, gs].astype(BF_NP),
            "wo": np.ascontiguousarray(_W["o"][gs, :]).astype(BF_NP),
            "mb": np.ascontiguousarray(mbias.astype(np.float32).reshape(TB, 128).T),
            "vones": ones,
        })
    return in_maps


_W = {}


def kernel(x_q, x_kv, pad_mask, Wq, Wk, Wv, Wo, bo):
    global LAST_RESULTS
    x_q = np.asarray(x_q, dtype=np.float32)
    x_kv = np.asarray(x_kv, dtype=np.float32)
    pad_mask = np.asarray(pad_mask)
    _W["q"] = np.asarray(Wq, dtype=np.float32)
    _W["k"] = np.asarray(Wk, dtype=np.float32)
    _W["v"] = np.asarray(Wv, dtype=np.float32)
    _W["o"] = np.asarray(Wo, dtype=np.float32)
    bo = np.asarray(bo, dtype=np.float32)

    nc = _get_compiled()
    in_maps = make_in_maps(x_q, x_kv, pad_mask)
    res = bass_utils.run_bass_kernel_spmd(nc, in_maps, list(range(N_CORES)))
    LAST_RESULTS = res
    outp = np.zeros((B, T, D), np.float32)
    for b in range(B):
        outp[b] = res.results[2 * b]["out"] + res.results[2 * b + 1]["out"]
    outp += bo[None, None, :]
    return outp


# revision 16
# speedup vs baseline: 1.2268x; 1.2268x over previous
"""Multi-head attention, tensor-parallel over heads x data-parallel over batch.

8 NeuronCores: core c handles batch b=c//2, head-group g=c%2 (8 heads, 512 chans).
Each core computes its head-group's attention + partial output projection;
the two partials per batch are summed on the host (row-parallel Wo unshard).

v3 (all-bf16 matmuls, continuous-PE schedule, DMA choreography):
  - every matmul operand is bf16 (halves DMA + LDWEIGHTS; PSUM stays fp32)
  - softmax 1/sum on VectorE via reciprocal_approx_fast (ScalarE runs ONLY
    Exp -> single ACT table set, no table switching, and the norm no longer
    clogs the ACT FIFO). recip inputs must be partition-offset-0 APs (the
    custom DVE op silently misreads offset sources), hence the [ones|v]
    stationary packing that puts sums on partitions 0:64.
  - merged multi-chunk input DMAs (few queue issues; per-chunk pacing) split
    across Sync and GpSimd queues
  - only pair0's first q/k chains run upfront; every other projection chain
    is interleaved into the attention jb-loops, so TensorE streams without
    gaps and the PE HAM stays un-throttled; out-proj chains for the first
    token half are interleaved into pair 3's attention
  - output DMAs round-robin over 4 queues

Per-core dataflow:
  qT/kT = W.T @ xT        [chan, tok] transposed projections
  v     = xkv @ Wv        [tok, chan] natural projection, stored per head
                          pair as [ones|v_even|ones|v_odd] (64 cols each)
  scoresT[j,i] = k.q      row-group packed pairs -> one [128,1024] psum
  expT  = exp(s*scale + maskbias)   one ScalarE op per (pair, ih, jb), bf16 out
  AV:   lhsT = [ones|v_h] (M=128) -> po rows = [sums replicated | o_h]
  norm: rb = recip(sums) on VectorE, oT = po * rb on VectorE (bf16 out)
  out   = oT.T @ Wo       partial output projection
"""

import numpy as np

import concourse.bacc as bacc
import concourse.mybir as mybir
import concourse.tile as tile
from concourse import bass_utils

B = 4
T = 1024          # tokens (N = L)
D = 1024          # model dim
CH = 64           # channels per head
G = 512           # channels per head-group (8 heads)
SCALE = CH ** -0.5
NEG = -30000.0    # mask bias (exp(x + NEG) == 0)
F32 = mybir.dt.float32
BF16 = mybir.dt.bfloat16
BF_NP = mybir.dt.np(mybir.dt.bfloat16)

N_CORES = 8
KB = 8            # 128-row contraction blocks over D
TB = 8            # 128-token blocks
PAIRS = 4         # head pairs per core
VW = 256          # v-tile columns per head pair: [ones|v_even|ones|v_odd]
AV_LAG = 3        # software-pipeline depth: AV trails scores/exp by this many jb

LAST_RESULTS = None
_CACHE = {}


def _emit(tc):
    nc = tc.nc
    xqT = nc.dram_tensor("xqT", [D, T], BF16, kind="ExternalInput").ap()
    xkvT = nc.dram_tensor("xkvT", [D, T], BF16, kind="ExternalInput").ap()
    wq = nc.dram_tensor("wq", [D, G], BF16, kind="ExternalInput").ap()
    wk = nc.dram_tensor("wk", [D, G], BF16, kind="ExternalInput").ap()
    wv = nc.dram_tensor("wv", [D, G], BF16, kind="ExternalInput").ap()
    wo = nc.dram_tensor("wo", [G, D], BF16, kind="ExternalInput").ap()
    mb = nc.dram_tensor("mb", [128, TB], F32, kind="ExternalInput").ap()
    vones = nc.dram_tensor("vones", [128, PAIRS * 2 * CH], BF16,
                           kind="ExternalInput").ap()
    out = nc.dram_tensor("out", [T, D], F32, kind="ExternalOutput").ap()

    Exp = mybir.ActivationFunctionType.Exp

    with (
        tc.tile_pool(name="wpool", bufs=1) as wpool,
        tc.tile_pool(name="xpool", bufs=1) as xpool,
        tc.tile_pool(name="apool", bufs=1) as apool,
        tc.tile_pool(name="epool", bufs=1) as epool,
        tc.tile_pool(name="npool", bufs=1) as npool,
        tc.tile_pool(name="opool", bufs=4) as opool,
        tc.tile_pool(name="psum", bufs=1, space="PSUM") as psum,
    ):
        # ---------------- input DMA ----------------
        # One dma_start rides ONE queue (~23 GB/s), so transfers are split
        # per 128KB piece and spread over the sync/scalar/gpsimd queues in
        # first-use order. Issue cost is ~0.6-0.9us per dma_start, so each
        # queue carries only what it must before its compute begins.
        mask_t = wpool.tile([128, TB], F32, name="mask_t", tag="mask")
        nc.sync.dma_start(mask_t[:], mb[:])

        xkv_b = xpool.tile([128, KB * T], BF16, name="xkv", tag="xkv")
        xq_b = xpool.tile([128, KB * T], BF16, name="xq", tag="xq")
        xkv_src = xkvT.rearrange("(kb p) t -> p kb t", p=128)
        xq_src = xqT.rearrange("(kb p) t -> p kb t", p=128)
        xkv_dst = xkv_b.rearrange("p (kb t) -> p kb t", t=T)
        xq_dst = xq_b.rearrange("p (kb t) -> p kb t", t=T)
        xkv_t = [xkv_b[:, k * T:(k + 1) * T] for k in range(KB)]
        xq_t = [xq_b[:, k * T:(k + 1) * T] for k in range(KB)]

        wv_b = wpool.tile([128, KB * G], BF16, name="wv", tag="wv")
        wq_b = wpool.tile([128, KB * G], BF16, name="wq", tag="wq")
        wk_b = wpool.tile([128, KB * G], BF16, name="wk", tag="wk")
        wv_t = [wv_b[:, k * G:(k + 1) * G] for k in range(KB)]
        wq_t = [wq_b[:, k * G:(k + 1) * G] for k in range(KB)]
        wk_t = [wk_b[:, k * G:(k + 1) * G] for k in range(KB)]
        wo_b = wpool.tile([128, PAIRS * D], BF16, name="wo", tag="wo")
        wo_t = [wo_b[:, m * D:(m + 1) * D] for m in range(PAIRS)]

        # sync: x_kv column-half A (tokens 0:512 of every chunk -- all that
        # v-chains tb0-3 and the kT ic0 chains read), then half B
        for k in range(KB):
            nc.sync.dma_start(xkv_dst[:, k, 0:512], xkv_src[:, k, 0:512])
        for k in range(KB):
            nc.sync.dma_start(xkv_dst[:, k, 512:1024],
                              xkv_src[:, k, 512:1024])

        # scalar: wv, wq, vones (all needed by ~15-20us; exps start later)
        wdst = {id(wv_b): wv_b.rearrange("p (kb g) -> p kb g", g=G),
                id(wq_b): wq_b.rearrange("p (kb g) -> p kb g", g=G),
                id(wk_b): wk_b.rearrange("p (kb g) -> p kb g", g=G)}
        wsrc = {id(wv_b): wv.rearrange("(kb p) g -> p kb g", p=128),
                id(wq_b): wq.rearrange("(kb p) g -> p kb g", p=128),
                id(wk_b): wk.rearrange("(kb p) g -> p kb g", p=128)}
        for k in range(KB):
            nc.scalar.dma_start(wdst[id(wv_b)][:, k, :], wsrc[id(wv_b)][:, k, :])
        for k in range(KB):
            nc.scalar.dma_start(wdst[id(wq_b)][:, k, :], wsrc[id(wq_b)][:, k, :])

        # v tiles: ones into slots 0 and 2 of each pair block (one DMA per
        # tb: the f=2,c=128 view's last-dim 0:64 slice hits cols {0:64,
        # 128:192} of each 256-col pair block)
        v_t = [apool.tile([128, PAIRS * VW], BF16, name=f"v{tb}",
                          tag=f"v{tb}") for tb in range(TB)]
        vo_src = vones.rearrange("p (pb f c) -> p pb f c", f=2, c=CH)
        for tb in range(TB):
            v2c = v_t[tb].rearrange("p (pb f c) -> p pb f c", f=2, c=128)
            nc.scalar.dma_start(v2c[:, :, :, 0:CH], vo_src[:])

        # gpsimd: xq half A, wk, xq half B, wo
        for k in range(KB):
            nc.gpsimd.dma_start(xq_dst[:, k, 0:512], xq_src[:, k, 0:512])
        for k in range(KB):
            nc.gpsimd.dma_start(wdst[id(wk_b)][:, k, :], wsrc[id(wk_b)][:, k, :])
        for k in range(KB):
            nc.gpsimd.dma_start(xq_dst[:, k, 512:1024],
                                xq_src[:, k, 512:1024])
        wo_dst = wo_b.rearrange("p (m d) -> p m d", d=D)
        wo_src2 = wo.rearrange("(m p) d -> p m d", p=128)
        for m in range(PAIRS):
            nc.gpsimd.dma_start(wo_dst[:, m, :], wo_src2[:, m, :])

        # ---------------- v = xkv @ Wv (natural layout) ----------------
        def v_chain(tb):
            ps = psum.tile([128, 512], F32, name="ps_acc", tag="acc", bufs=2)
            for k in range(KB):
                nc.tensor.matmul(
                    ps[:],
                    xkv_t[k][:, tb * 128:(tb + 1) * 128],
                    wv_t[k][:],
                    start=(k == 0),
                    stop=(k == KB - 1),
                )
            v4 = v_t[tb].rearrange("p (pb four c) -> p pb four c",
                                   four=4, c=CH)
            ps3 = ps.rearrange("p (pb two c) -> p pb two c", two=2, c=CH)
            nc.vector.tensor_copy(v4[:, :, 1:2, :], ps3[:, :, 0:1, :])
            nc.vector.tensor_copy(v4[:, :, 3:4, :], ps3[:, :, 1:2, :])

        # v tiles for the first half of j (all attention(p0, jb0..3) needs);
        # tb 4-7 are interleaved into p0-ih0 so attention starts sooner
        for tb in range(4):
            v_chain(tb)

        # ---------------- qT/kT projection chains ----------------
        qT_t = [apool.tile([128, T], BF16, name=f"qT{m}", tag=f"qT{m}")
                for m in range(PAIRS)]
        kT_t = [apool.tile([128, T], BF16, name=f"kT{m}", tag=f"kT{m}")
                for m in range(PAIRS)]

        def proj_chain(p, which, ic):
            src, w_t, dst = ((xq_t, wq_t, qT_t), (xkv_t, wk_t, kT_t))[which]
            csl = slice(ic * 512, (ic + 1) * 512)
            ps = psum.tile([128, 512], F32, name="ps_acc", tag="acc", bufs=2)
            for k in range(KB):
                nc.tensor.matmul(
                    ps[:],
                    w_t[k][:, p * 128:(p + 1) * 128],
                    src[k][:, csl],
                    start=(k == 0),
                    stop=(k == KB - 1),
                )
            nc.vector.tensor_copy(dst[p][:, csl], ps[:])

        # out-projection chain; tb 0..3 read only the ih=0 half of oT, so
        # they can interleave into pair 3's attention
        oq_cycle = [None, None, None]  # filled below (queues)

        def out_chain(tb, ncx, oT_t):
            tsl = slice(tb * 128, (tb + 1) * 128)
            nsl = slice(ncx * 512, (ncx + 1) * 512)
            ps = psum.tile([128, 512], F32, name="ps_acc", tag="acc", bufs=2)
            for m in range(PAIRS):
                nc.tensor.matmul(
                    ps[:],
                    oT_t[m][:, tsl],
                    wo_t[m][:, nsl],
                    start=(m == 0),
                    stop=(m == PAIRS - 1),
                )
            ot = opool.tile([128, 512], F32, name="ot", tag="ot")
            nc.vector.tensor_copy(ot[:], ps[:])
            # two half-DMAs on different queues: halves the drain time of
            # the last tile, which is pure tail latency
            for hf in (0, 1):
                q = oq_cycle[(tb * 4 + ncx * 2 + hf) % 3]
                hsl = slice(nsl.start + hf * 256, nsl.start + (hf + 1) * 256)
                q.dma_start(out[tsl, hsl], ot[:, hf * 256:(hf + 1) * 256])

        # upfront: only what attention(p0, ih0, jb0..3) needs
        proj_chain(0, 0, 0)   # q0 ic0
        proj_chain(0, 1, 0)   # k0 ic0

        # ---------------- attention ----------------
        # fill[(p, ih)]: chain thunks run at jb==2 and jb==5 -- the chain a
        # scores MM depends on is always EMITTED earlier, and the Tensor
        # queue executes in order, so this can never deadlock.
        oT_t = [apool.tile([128, T], BF16, name=f"oT{m}", tag=f"oT{m}")
                for m in range(PAIRS)]
        oq_cycle[:] = [nc.scalar, nc.sync, nc.gpsimd]
        # fill[(p, ih)]: {jb: [thunk specs]} run after that jb's scores/exp.
        # The chain a later scores/AV MM depends on is always EMITTED
        # earlier, and the Tensor queue executes in order -> no deadlock.
        fill = {
            (0, 0): {1: [("v", 4), ("v", 5)], 2: [(0, 1, 1)],
                     3: [("v", 6), ("v", 7)], 5: [(0, 0, 1)]},
            (0, 1): {2: [(1, 0, 0)], 5: [(1, 1, 0)]},
            (1, 0): {2: [(1, 1, 1)], 5: [(1, 0, 1)]},
            (1, 1): {2: [(2, 0, 0)], 5: [(2, 1, 0)]},
            (2, 0): {2: [(2, 1, 1)], 5: [(2, 0, 1)]},
            (2, 1): {2: [(3, 0, 0)], 5: [(3, 1, 0)]},
            (3, 0): {2: [(3, 1, 1)], 5: [(3, 0, 1)]},
            (3, 1): {2: [("out", 0, 0)], 5: [("out", 0, 1)]},
        }
        for p in range(PAIRS):
            oT = oT_t[p]
            for ih in range(2):
                isl = slice(ih * 512, (ih + 1) * 512)
                po = psum.tile([128, 1024], F32, name="po", tag="po", bufs=1)
                pend = []
                slots = fill[(p, ih)]
                for jb in range(TB):
                    jsl = slice(jb * 128, (jb + 1) * 128)
                    # one [128, 1024] psum tile: bank0 = head 2p, bank1 = 2p+1
                    pss = psum.tile([128, 1024], F32, name="ps_s", tag="sc",
                                    bufs=2)
                    for h in (0, 1):
                        hsl = slice(h * 64, (h + 1) * 64)
                        nc.tensor.matmul(
                            pss[:, h * 512:(h + 1) * 512],
                            kT_t[p][hsl, jsl],
                            qT_t[p][hsl, isl],
                        )
                    # one exp for both heads; mask bias is per-partition (= j)
                    et = epool.tile([128, 1024], BF16, name="et", tag="et",
                                    bufs=6)
                    nc.scalar.activation(et[:], pss[:], Exp,
                                         bias=mask_t[:, jb:jb + 1],
                                         scale=SCALE)
                    pend.append((jb, et))
                    if len(pend) > AV_LAG:
                        _av(nc, pend.pop(0), p, po, v_t)
                    for s in slots.get(jb, ()):
                        if s[0] == "v":
                            v_chain(s[1])
                        elif s[0] == "out":
                            out_chain(s[1], s[2], oT_t)
                        else:
                            proj_chain(*s)
                while pend:
                    _av(nc, pend.pop(0), p, po, v_t)
                # normalize. po bank h = [s_h (64 rows) | o_h (64 rows)]
                # rb = 1/s on VectorE (~51 ULP approx; sums are O(1..1e3));
                # recip inputs MUST be partition-offset-0 APs
                rb_e = npool.tile([CH, 512], F32, name="rb_e", tag="rb",
                                  bufs=4)
                nc.vector.reciprocal_approx_fast(rb_e[:], po[0:CH, 0:512])
                nc.vector.tensor_mul(oT[0:CH, isl], po[CH:128, 0:512],
                                     rb_e[:])
                rb_o = npool.tile([CH, 512], F32, name="rb_o", tag="rb",
                                  bufs=4)
                nc.vector.reciprocal_approx_fast(rb_o[:], po[0:CH, 512:1024])
                nc.vector.tensor_mul(oT[CH:128, isl], po[CH:128, 512:1024],
                                     rb_o[:])

        # ---------------- out = oT.T @ Wo (rest) ----------------
        # tb 1..3 need only ih0 norms (all done); tb 4..7 need the p3-ih1
        # norm and run while VectorE finishes it
        for tb in range(1, TB):
            for ncx in range(2):
                out_chain(tb, ncx, oT_t)


def _av(nc, item, p, po, v_t):
    jb, et = item
    for h in (0, 1):
        # head 2p+h stationary: [ones|v_h] -> out rows [sums | o_h]
        csl = slice(p * VW + h * 128, p * VW + h * 128 + 128)
        nc.tensor.matmul(
            po[:, h * 512:(h + 1) * 512],
            v_t[jb][:, csl],
            et[:, h * 512:(h + 1) * 512],
            start=(jb == 0),
            stop=(jb == TB - 1),
        )


def build_nc():
    nc = bacc.Bacc("TRN2", target_bir_lowering=False, debug=False,
                   num_devices=N_CORES)
    with tile.TileContext(nc) as tc:
        _emit(tc)
    nc.compile()
    return nc


def _get_compiled():
    if "nc" not in _CACHE:
        _CACHE["nc"] = build_nc()
    return _CACHE["nc"]


def make_in_maps(x_q, x_kv, pad_mask):
    ones = np.ones((128, PAIRS * 2 * CH), BF_NP)
    in_maps = []
    for c in range(N_CORES):
        b, g = divmod(c, 2)
        gs = slice(g * G, (g + 1) * G)
        mbias = np.where(pad_mask[b], np.float32(NEG), np.float32(0.0))
        in_maps.append({
            "xqT": np.ascontiguousarray(x_q[b].T).astype(BF_NP),
            "xkvT": np.ascontiguousarray(x_kv[b].T).astype(BF_NP),
            "wq": _W["q"][:, gs].astype(BF_NP),
            "wk": _W["k"][:, gs].astype(BF_NP),
            "wv": _W["v"][:, gs].astype(BF_NP),
            "wo": np.ascontiguousarray(_W["o"][gs, :]).astype(BF_NP),
            "mb": np.ascontiguousarray(mbias.astype(np.float32).reshape(TB, 128).T),
            "vones": ones,
        })
    return in_maps


_W = {}


def kernel(x_q, x_kv, pad_mask, Wq, Wk, Wv, Wo, bo):
    global LAST_RESULTS
    x_q = np.asarray(x_q, dtype=np.float32)
    x_kv = np.asarray(x_kv, dtype=np.float32)
    pad_mask = np.asarray(pad_mask)
    _W["q"] = np.asarray(Wq, dtype=np.float32)
    _W["k"] = np.asarray(Wk, dtype=np.float32)
    _W["v"] = np.asarray(Wv, dtype=np.float32)
    _W["o"] = np.asarray(Wo, dtype=np.float32)
    bo = np.asarray(bo, dtype=np.float32)

    nc = _get_compiled()
    in_maps = make_in_maps(x_q, x_kv, pad_mask)
    res = bass_utils.run_bass_kernel_spmd(nc, in_maps, list(range(N_CORES)))
    LAST_RESULTS = res
    outp = np.zeros((B, T, D), np.float32)
    for b in range(B):
        outp[b] = res.results[2 * b]["out"] + res.results[2 * b + 1]["out"]
    outp += bo[None, None, :]
    return outp
